# revision 5
# baseline (speedup 1.0000x reference)
# Sparse multi-head attention + output projection on 8 Trainium2 cores.
#
# Problem (hardcoded): q [2,16,2048,64], k/v [2,16,4096,64], W [1024,1024],
# b [1024], memory_length scalar. Mask: row r attends cols t < L(r),
# L(r) = min(KV, max(r, m-1) + C + 1), C = KV - S.
#
# Sharding: 4 cores per batch; core j (of its batch) owns query blocks
# {j, 4+j, 8+j, 12+j} (128 rows each) for ALL 16 heads. One SPMD program
# (uniform union schedule); per-core sparsity differences are data-driven
# via 0/1 mask tiles. Per head: scores^T tiles [t=128, q<=512] via f32r
# matmul, exp on ScalarE (PSUM->SBUF, grouped 3 chunks/call), PV matmul
# with a ones-column appended to V giving softmax denominators for free,
# normalize via reciprocal + partition_broadcast, then a column-parallel
# output projection from SBUF-resident attn^T.
import numpy as np

B, H, S, Dh = 2, 16, 2048, 64
KV = 4096
HID = H * Dh
NBLK = S // 128          # 16 query blocks per batch
POS = 4                  # block positions per core
CORES = 8
SCALE = 1.0 / np.sqrt(Dh).astype(np.float32)
GROUP = 3                # score chunks per exp call (3 PSUM banks)

_prog_cache = {}


def _limits(m):
    # L(r) for all rows of a batch
    r = np.arange(S)
    return np.minimum(KV, np.maximum(r, m - 1) + (KV - S) + 1)


def _build_schedule(m):
    L = _limits(m)
    # block k rows [128k, 128k+128)
    Lmin = np.array([L[128 * k] for k in range(NBLK)])
    Lmax = np.array([L[128 * k + 127] for k in range(NBLK)])
    full_until = Lmin // 128 - 1          # chunks 0..full_until fully allowed
    active_until = (Lmax + 127) // 128 - 1
    # core j position p -> block 4p + j
    blk = lambda j, p: 4 * p + j
    union_until = [max(active_until[blk(j, p)] for j in range(4)) for p in range(POS)]
    n_chunks = max(union_until) + 1

    def p0(c):
        for p in range(POS):
            if union_until[p] >= c:
                return p
        return POS

    qoff = [128 * p0(c) for c in range(n_chunks)]
    # mask regions: (c, p) active in union where any core is not-full
    regions = []   # (c, p)
    for c in range(n_chunks):
        for p in range(p0(c), POS):
            if any(c > full_until[blk(j, p)] for j in range(4)):
                regions.append((c, p))
    # per-core mask data
    masks = np.empty((4, max(1, len(regions)), 128, 128), np.float32)
    masks[:] = 1.0
    for mi, (c, p) in enumerate(regions):
        for j in range(4):
            rows = 128 * blk(j, p) + np.arange(128)
            cols = 128 * c + np.arange(128)
            # score tiles are transposed: [t_local (partition), r_local (free)]
            masks[j, mi] = (cols[:, None] < L[rows][None, :]).astype(np.float32)
    groups = []
    for g0 in range(0, n_chunks, GROUP):
        cs = list(range(g0, min(g0 + GROUP, n_chunks)))
        groups.append(cs)
    return dict(n_chunks=n_chunks, qoff=qoff, regions=regions, masks=masks,
                groups=groups, blocks=[[blk(j, p) for p in range(POS)] for j in range(4)])


def _build_program(sched, n_masks):
    import concourse.bacc as bacc
    import concourse.mybir as mybir
    import concourse.tile as tile

    f32 = mybir.dt.float32
    f32r = mybir.dt.float32r
    Exp = mybir.ActivationFunctionType.Exp
    n_chunks = sched["n_chunks"]
    qoff = sched["qoff"]
    groups = sched["groups"]
    reg_idx = {cp: i for i, cp in enumerate(sched["regions"])}

    nc = bacc.Bacc("TRN2", target_bir_lowering=False, debug=False)
    kt_d = nc.dram_tensor("kt", [H, Dh, KV], f32r, kind="ExternalInput").ap()
    v_d = nc.dram_tensor("v", [H, 128, KV // 128, Dh + 1], f32r, kind="ExternalInput").ap()
    qt_d = nc.dram_tensor("qt", [H, Dh, 512], f32r, kind="ExternalInput").ap()
    wt_d = nc.dram_tensor("wt", [HID, HID], f32r, kind="ExternalInput").ap()
    bb_d = nc.dram_tensor("bb", [128, HID], f32, kind="ExternalInput").ap()
    mk_d = nc.dram_tensor("masks", [n_masks, 128, 128], f32, kind="ExternalInput").ap()
    out_d = nc.dram_tensor("out", [512, HID], f32, kind="ExternalOutput").ap()

    with tile.TileContext(nc) as tc:
        with (
            tc.tile_pool(name="consts", bufs=1) as consts,
            tc.tile_pool(name="attn", bufs=1) as attn_pool,
        ):
            bb_sb = consts.tile([128, HID], f32)
            nc.sync.dma_start(out=bb_sb, in_=bb_d)
            mk_sb = consts.tile([128, n_masks, 128], f32)
            nc.sync.dma_start(out=mk_sb, in_=mk_d.rearrange("n p t -> p n t"))
            attnT = attn_pool.tile([Dh, H, 512], f32r)

            with (
                tc.tile_pool(name="kv", bufs=2) as kv_pool,
                tc.tile_pool(name="pt", bufs=3) as pt_pool,
                tc.tile_pool(name="small", bufs=2) as small,
                tc.tile_pool(name="ps_s", bufs=2, space="PSUM") as ps_s,
                tc.tile_pool(name="ps_av", bufs=2, space="PSUM") as ps_av,
            ):
                for h in range(H):
                    kt_h = kv_pool.tile([Dh, KV], f32r, tag="kt")
                    nc.sync.dma_start(out=kt_h, in_=kt_d[h])
                    v_h = kv_pool.tile([128, KV // 128, Dh + 1], f32r, tag="v")
                    nc.sync.dma_start(out=v_h, in_=v_d[h])
                    qt_h = kv_pool.tile([Dh, 512], f32r, tag="qt")
                    nc.sync.dma_start(out=qt_h, in_=qt_d[h])

                    av = ps_av.tile([Dh + 1, 512], f32, tag="av")
                    for cs in groups:
                        sg = ps_s.tile([128, GROUP, 512], f32, tag="sg")
                        for ci, c in enumerate(cs):
                            nc.tensor.matmul(
                                sg[:, ci, qoff[c]:512],
                                kt_h[:, 128 * c:128 * (c + 1)],
                                qt_h[:, qoff[c]:512],
                                start=True, stop=True,
                            )
                        ptg = pt_pool.tile([128, GROUP, 512], f32r, tag="ptg")
                        ci0 = 0
                        while ci0 < len(cs):
                            ci1 = ci0
                            while ci1 < len(cs) and qoff[cs[ci1]] == qoff[cs[ci0]]:
                                ci1 += 1
                            qg = qoff[cs[ci0]]
                            nc.scalar.activation(
                                ptg[:, ci0:ci1, qg:512], sg[:, ci0:ci1, qg:512],
                                Exp, scale=float(SCALE),
                            )
                            ci0 = ci1
                        for ci, c in enumerate(cs):
                            for p in range(POS):
                                mi = reg_idx.get((c, p))
                                if mi is not None:
                                    nc.vector.tensor_mul(
                                        ptg[:, ci, 128 * p:128 * (p + 1)],
                                        ptg[:, ci, 128 * p:128 * (p + 1)],
                                        mk_sb[:, mi, :],
                                    )
                            nc.tensor.matmul(
                                av[:, qoff[c]:512],
                                v_h[:, c, :],
                                ptg[:, ci, qoff[c]:512],
                                start=(c == 0), stop=(c == n_chunks - 1),
                            )
                    recip = small.tile([1, 512], f32, tag="recip")
                    nc.vector.reciprocal(recip, av[Dh:Dh + 1, :])
                    mult = small.tile([Dh, 512], f32, tag="mult")
                    nc.gpsimd.partition_broadcast(mult, recip)
                    nc.vector.tensor_mul(attnT[:, h, :], av[0:Dh, :], mult)

            # output projection: out[q, :] = attn @ W.T + b
            with (
                tc.tile_pool(name="wt", bufs=3) as wt_pool,
                tc.tile_pool(name="ps_o", bufs=1, space="PSUM") as ps_o,
                tc.tile_pool(name="osb", bufs=2) as osb_pool,
            ):
                o_tiles = [[ps_o.tile([128, 512], f32, tag=f"o{qs}{jh}",
                                      name=f"o{qs}{jh}")
                            for jh in range(2)] for qs in range(4)]
                for h in range(H):
                    wt_h = wt_pool.tile([Dh, HID], f32r, tag="wt")
                    nc.sync.dma_start(out=wt_h, in_=wt_d[Dh * h:Dh * (h + 1), :])
                    for qs in range(4):
                        for jh in range(2):
                            nc.tensor.matmul(
                                o_tiles[qs][jh],
                                attnT[:, h, 128 * qs:128 * (qs + 1)],
                                wt_h[:, 512 * jh:512 * (jh + 1)],
                                start=(h == 0), stop=(h == H - 1),
                            )
                for qs in range(4):
                    ob = osb_pool.tile([128, HID], f32, tag="ob")
                    for jh in range(2):
                        nc.vector.tensor_add(
                            ob[:, 512 * jh:512 * (jh + 1)], o_tiles[qs][jh],
                            bb_sb[:, 512 * jh:512 * (jh + 1)])
                    nc.sync.dma_start(out=out_d[128 * qs:128 * (qs + 1), :], in_=ob)

    nc.compile()
    return nc


def _prepare(q, k, v, m, W, b):
    sched = _build_schedule(m)
    kt = np.ascontiguousarray(k.transpose(0, 1, 3, 2))          # [B,H,64,KV]
    vr = v.reshape(B, H, KV // 128, 128, Dh).transpose(0, 1, 3, 2, 4)
    v_aug = np.empty((B, H, 128, KV // 128, Dh + 1), np.float32)
    v_aug[..., :Dh] = vr
    v_aug[..., Dh] = 1.0
    wt = np.ascontiguousarray(W.T)
    bb = np.ascontiguousarray(np.broadcast_to(b, (128, HID)))
    in_maps = []
    for core in range(CORES):
        bidx, j = core // 4, core % 4
        rows = np.concatenate([128 * kblk + np.arange(128) for kblk in sched["blocks"][j]])
        qt = np.ascontiguousarray(q[bidx][:, rows, :].transpose(0, 2, 1))  # [H,64,512]
        in_maps.append({
            "kt": kt[bidx], "v": v_aug[bidx], "qt": qt, "wt": wt, "bb": bb,
            "masks": np.ascontiguousarray(sched["masks"][j]),
        })
    return sched, in_maps


def kernel(q, k, v, memory_length, W, b, _trace=False, _trace_cores=None):
    from concourse.bass_utils import run_bass_kernel_spmd

    q = np.asarray(q, np.float32)
    k = np.asarray(k, np.float32)
    v = np.asarray(v, np.float32)
    W = np.asarray(W, np.float32)
    b = np.asarray(b, np.float32)
    m = int(memory_length)

    sched, in_maps = _prepare(q, k, v, m, W, b)
    key = (m, sched["masks"].shape[1])
    if key not in _prog_cache:
        _prog_cache[key] = _build_program(sched, sched["masks"].shape[1])
    nc = _prog_cache[key]

    res = run_bass_kernel_spmd(
        nc, in_maps, list(range(CORES)),
        trace=_trace, trace_cores=_trace_cores,
    )
    out = np.empty((B, S, HID), np.float32)
    for core in range(CORES):
        bidx, j = core // 4, core % 4
        oc = res.results[core]["out"]
        for p, kblk in enumerate(sched["blocks"][j]):
            out[bidx, 128 * kblk:128 * (kblk + 1)] = oc[128 * p:128 * (p + 1)]

    L = _limits(m)
    mask2d = (np.arange(KV)[None, :] >= L[:, None]).astype(np.float32)
    mask_full = np.broadcast_to(mask2d, (B, H, S, KV)).copy()
    if _trace:
        return (out, mask_full), res
    return out, mask_full


# revision 8
# speedup vs baseline: 1.2848x; 1.2848x over previous
# Sparse multi-head attention + output projection on 8 Trainium2 cores.
#
# Problem (hardcoded): q [2,16,2048,64], k/v [2,16,4096,64], W [1024,1024],
# b [1024], memory_length scalar. Mask: row r attends cols t < L(r),
# L(r) = min(KV, max(r, m-1) + C + 1), C = KV - S.
#
# Sharding: 4 cores per batch; core j (of its batch) owns query blocks
# {j, 4+j, 8+j, 12+j} (128 rows each) for ALL 16 heads. One SPMD program
# (uniform union schedule); per-core sparsity differences are data-driven
# via 0/1 mask tiles. Per head: scores^T tiles [t=128, q<=512] via fp16
# matmul (heads row-packed in pairs on the 128x128 PE array), exp on
# ScalarE (PSUM->SBUF, grouped chunks per call), PV matmul with a
# ones-column appended to V giving softmax denominators for free,
# normalize via fast reciprocal + partition_broadcast, then a
# column-parallel output projection from SBUF-resident attn^T.
import numpy as np
MM_NP = np.float16
B, H, S, Dh = 2, 16, 2048, 64
KV = 4096
HID = H * Dh
NBLK = S // 128          # 16 query blocks per batch
POS = 4                  # block positions per core
CORES = 8
SCALE = 1.0 / np.sqrt(Dh).astype(np.float32)
GROUP = 3                # score chunks per exp call (3 PSUM banks)

_prog_cache = {}


def _limits(m):
    # L(r) for all rows of a batch
    r = np.arange(S)
    return np.minimum(KV, np.maximum(r, m - 1) + (KV - S) + 1)


def _build_schedule(m):
    L = _limits(m)
    Lmin = np.array([L[128 * k] for k in range(NBLK)])
    Lmax = np.array([L[128 * k + 127] for k in range(NBLK)])
    full_until = Lmin // 128 - 1          # chunks 0..full_until fully allowed
    active_until = (Lmax + 127) // 128 - 1
    blk = lambda j, p: 4 * p + j          # core j, position p -> block
    union_until = [max(active_until[blk(j, p)] for j in range(4)) for p in range(POS)]
    n_chunks = max(union_until) + 1

    def p0(c):
        for p in range(POS):
            if union_until[p] >= c:
                return p
        return POS

    qoff = [128 * p0(c) for c in range(n_chunks)]
    regions = []   # (c, p) needing a data mask
    for c in range(n_chunks):
        for p in range(p0(c), POS):
            if any(c > full_until[blk(j, p)] for j in range(4)):
                regions.append((c, p))
    masks = np.empty((4, max(1, len(regions)), 128, 128), np.float32)
    masks[:] = 1.0
    for mi, (c, p) in enumerate(regions):
        for j in range(4):
            rows = 128 * blk(j, p) + np.arange(128)
            cols = 128 * c + np.arange(128)
            # score tiles are transposed: [t_local (partition), r_local (free)]
            masks[j, mi] = (cols[:, None] < L[rows][None, :]).astype(np.float32)
    groups = [list(range(g0, min(g0 + GROUP, n_chunks)))
              for g0 in range(0, n_chunks, GROUP)]
    return dict(n_chunks=n_chunks, qoff=qoff, regions=regions, masks=masks,
                groups=groups, blocks=[[blk(j, p) for p in range(POS)] for j in range(4)])


def _build_program(sched, n_masks):
    import concourse.bacc as bacc
    import concourse.mybir as mybir
    import concourse.tile as tile

    f32 = mybir.dt.float32
    bfl = mybir.dt.float16
    Exp = mybir.ActivationFunctionType.Exp
    n_chunks = sched["n_chunks"]
    qoff = sched["qoff"]
    groups = sched["groups"]
    reg_idx = {cp: i for i, cp in enumerate(sched["regions"])}

    nc = bacc.Bacc("TRN2", target_bir_lowering=False, debug=False)
    # head pairs (2p, 2p+1) stacked on the partition axis for QK row-packing
    kt_d = nc.dram_tensor("kt", [H // 2, 128, KV], bfl, kind="ExternalInput").ap()
    v_d = nc.dram_tensor("v", [H, 128, KV // 128, Dh + 1], bfl, kind="ExternalInput").ap()
    qt_d = nc.dram_tensor("qt", [H // 2, 128, 512], bfl, kind="ExternalInput").ap()
    wt_d = nc.dram_tensor("wt", [HID, HID], bfl, kind="ExternalInput").ap()
    bb_d = nc.dram_tensor("bb", [128, HID], f32, kind="ExternalInput").ap()
    mk_d = nc.dram_tensor("masks", [n_masks, 128, 128], bfl, kind="ExternalInput").ap()
    out_d = nc.dram_tensor("out", [512, HID], f32, kind="ExternalOutput").ap()

    with tile.TileContext(nc) as tc:
        with (
            tc.tile_pool(name="consts", bufs=1) as consts,
            tc.tile_pool(name="attn", bufs=1) as attn_pool,
        ):
            bb_sb = consts.tile([128, HID], f32)
            nc.sync.dma_start(out=bb_sb, in_=bb_d)
            mk_sb = consts.tile([128, n_masks, 128], bfl)
            nc.sync.dma_start(out=mk_sb, in_=mk_d.rearrange("n p t -> p n t"))
            attnT = attn_pool.tile([Dh, H, 512], bfl)

            with (
                tc.tile_pool(name="kv", bufs=2) as kv_pool,
                tc.tile_pool(name="pt", bufs=2) as pt_pool,
                tc.tile_pool(name="small", bufs=2) as small,
                tc.tile_pool(name="ps_s", bufs=1, space="PSUM") as ps_s,
                tc.tile_pool(name="ps_av", bufs=1, space="PSUM") as ps_av,
            ):
                for pr in range(H // 2):
                    heads = (2 * pr, 2 * pr + 1)
                    kt_p = kv_pool.tile([128, KV], bfl, tag="kt")
                    nc.sync.dma_start(out=kt_p, in_=kt_d[pr])
                    qt_p = kv_pool.tile([128, 512], bfl, tag="qt")
                    nc.sync.dma_start(out=qt_p, in_=qt_d[pr])
                    v_t = [None, None]
                    for hi, h in enumerate(heads):
                        v_t[hi] = kv_pool.tile([128, KV // 128, Dh + 1], bfl,
                                               tag=f"v{hi}", name=f"v{hi}")
                        nc.sync.dma_start(out=v_t[hi], in_=v_d[h])

                    av = [ps_av.tile([Dh + 1, 512], f32, tag=f"av{hi}", name=f"av{hi}")
                          for hi in range(2)]
                    for cs in groups:
                        sg = [ps_s.tile([128, GROUP, 512], f32, tag=f"sg{hi}",
                                        name=f"sg{hi}") for hi in range(2)]
                        for ci, c in enumerate(cs):
                            for hi in range(2):
                                nc.tensor.matmul(
                                    sg[hi][:, ci, qoff[c]:512],
                                    kt_p[64 * hi:64 * (hi + 1), 128 * c:128 * (c + 1)],
                                    qt_p[64 * hi:64 * (hi + 1), qoff[c]:512],
                                    start=True, stop=True,
                                    tile_position=(64 * hi, 0),
                                )
                        ptg = [pt_pool.tile([128, GROUP, 512], bfl, tag=f"ptg{hi}",
                                            name=f"ptg{hi}") for hi in range(2)]
                        for hi in range(2):
                            ci0 = 0
                            while ci0 < len(cs):
                                ci1 = ci0
                                while ci1 < len(cs) and qoff[cs[ci1]] == qoff[cs[ci0]]:
                                    ci1 += 1
                                qg = qoff[cs[ci0]]
                                nc.scalar.activation(
                                    ptg[hi][:, ci0:ci1, qg:512], sg[hi][:, ci0:ci1, qg:512],
                                    Exp, scale=float(SCALE),
                                )
                                ci0 = ci1
                        for hi in range(2):
                            for ci, c in enumerate(cs):
                                for p in range(POS):
                                    mi = reg_idx.get((c, p))
                                    if mi is not None:
                                        nc.vector.tensor_mul(
                                            ptg[hi][:, ci, 128 * p:128 * (p + 1)],
                                            ptg[hi][:, ci, 128 * p:128 * (p + 1)],
                                            mk_sb[:, mi, :],
                                        )
                                nc.tensor.matmul(
                                    av[hi][:, qoff[c]:512],
                                    v_t[hi][:, c, :],
                                    ptg[hi][:, ci, qoff[c]:512],
                                    start=(c == 0), stop=(c == n_chunks - 1),
                                )
                    for hi, h in enumerate(heads):
                        den = small.tile([1, 512], f32, tag=f"den{hi}",
                                         name=f"den{hi}")
                        nc.vector.tensor_copy(den, av[hi][Dh:Dh + 1, :])
                        recip = small.tile([1, 512], f32, tag=f"recip{hi}",
                                           name=f"recip{hi}")
                        nc.vector.reciprocal_approx_fast(recip, den)
                        mult = small.tile([Dh, 512], f32, tag=f"mult{hi}",
                                          name=f"mult{hi}")
                        nc.gpsimd.partition_broadcast(mult, recip)
                        nc.vector.tensor_mul(attnT[:, h, :], av[hi][0:Dh, :], mult)

            # output projection: out[q, :] = attn @ W.T + b
            with (
                tc.tile_pool(name="wt", bufs=3) as wt_pool,
                tc.tile_pool(name="ps_o", bufs=1, space="PSUM") as ps_o,
                tc.tile_pool(name="osb", bufs=2) as osb_pool,
            ):
                o_tiles = [[ps_o.tile([128, 512], f32, tag=f"o{qs}{jh}",
                                      name=f"o{qs}{jh}")
                            for jh in range(2)] for qs in range(4)]
                for h in range(H):
                    wt_h = wt_pool.tile([Dh, HID], bfl, tag="wt")
                    nc.sync.dma_start(out=wt_h, in_=wt_d[Dh * h:Dh * (h + 1), :])
                    for qs in range(4):
                        for jh in range(2):
                            nc.tensor.matmul(
                                o_tiles[qs][jh],
                                attnT[:, h, 128 * qs:128 * (qs + 1)],
                                wt_h[:, 512 * jh:512 * (jh + 1)],
                                start=(h == 0), stop=(h == H - 1),
                            )
                for qs in range(4):
                    ob = osb_pool.tile([128, HID], f32, tag="ob")
                    for jh in range(2):
                        nc.vector.tensor_add(
                            ob[:, 512 * jh:512 * (jh + 1)], o_tiles[qs][jh],
                            bb_sb[:, 512 * jh:512 * (jh + 1)])
                    nc.sync.dma_start(out=out_d[128 * qs:128 * (qs + 1), :], in_=ob)

    nc.compile()
    return nc


def _prepare(q, k, v, m, W, b):
    sched = _build_schedule(m)
    kt = np.ascontiguousarray(k.transpose(0, 1, 3, 2).astype(MM_NP))   # [B,H,64,KV]
    kt = kt.reshape(B, H // 2, 128, KV)
    vr = v.reshape(B, H, KV // 128, 128, Dh).transpose(0, 1, 3, 2, 4)
    v_aug = np.empty((B, H, 128, KV // 128, Dh + 1), MM_NP)
    v_aug[..., :Dh] = vr.astype(MM_NP)
    v_aug[..., Dh] = 1.0
    wt = np.ascontiguousarray(W.T.astype(MM_NP))
    bb = np.ascontiguousarray(np.broadcast_to(b, (128, HID)).astype(np.float32))
    in_maps = []
    for core in range(CORES):
        bidx, j = core // 4, core % 4
        rows = np.concatenate([128 * kblk + np.arange(128) for kblk in sched["blocks"][j]])
        qt = np.ascontiguousarray(
            q[bidx][:, rows, :].transpose(0, 2, 1).astype(MM_NP)).reshape(H // 2, 128, 512)
        in_maps.append({
            "kt": kt[bidx], "v": v_aug[bidx], "qt": qt, "wt": wt, "bb": bb,
            "masks": np.ascontiguousarray(sched["masks"][j].astype(MM_NP)),
        })
    return sched, in_maps


def kernel(q, k, v, memory_length, W, b, _trace=False, _trace_cores=None):
    from concourse.bass_utils import run_bass_kernel_spmd

    q = np.asarray(q, np.float32)
    k = np.asarray(k, np.float32)
    v = np.asarray(v, np.float32)
    W = np.asarray(W, np.float32)
    b = np.asarray(b, np.float32)
    m = int(memory_length)

    sched, in_maps = _prepare(q, k, v, m, W, b)
    key = (m, sched["masks"].shape[1])
    if key not in _prog_cache:
        _prog_cache[key] = _build_program(sched, sched["masks"].shape[1])
    nc = _prog_cache[key]

    res = run_bass_kernel_spmd(
        nc, in_maps, list(range(CORES)),
        trace=_trace, trace_cores=_trace_cores,
    )
    out = np.empty((B, S, HID), np.float32)
    for core in range(CORES):
        bidx, j = core // 4, core % 4
        oc = res.results[core]["out"]
        for p, kblk in enumerate(sched["blocks"][j]):
            out[bidx, 128 * kblk:128 * (kblk + 1)] = oc[128 * p:128 * (p + 1)]

    L = _limits(m)
    mask2d = (np.arange(KV)[None, :] >= L[:, None]).astype(np.float32)
    mask_full = np.broadcast_to(mask2d, (B, H, S, KV))
    if _trace:
        return (out, mask_full), res
    return out, mask_full


# revision 14
# speedup vs baseline: 1.7061x; 1.3279x over previous
# Sparse multi-head attention + output projection on 8 Trainium2 cores.
#
# Problem (hardcoded): q [2,16,2048,64], k/v [2,16,4096,64], W [1024,1024],
# b [1024], memory_length scalar. Mask: row r attends cols t < L(r),
# L(r) = min(KV, max(r, m-1) + C + 1), C = KV - S.
#
# Sharding: 4 cores per batch; core j (of its batch) owns query blocks
# {j, 4+j, 8+j, 12+j} (128 rows each) for ALL 16 heads. One SPMD program
# (uniform union schedule); per-core sparsity differences are data-driven
# via 0/1 mask tiles. Per head: scores^T tiles [t=128, q<=512] via fp16
# matmul (heads row-packed in pairs on the 128x128 PE array), exp on
# ScalarE (PSUM->SBUF, grouped chunks per call), PV matmul with a
# ones-column appended to V giving softmax denominators for free,
# normalize via fast reciprocal + partition_broadcast, then a
# column-parallel output projection from SBUF-resident attn^T.
import numpy as np
MM_NP = np.float16
B, H, S, Dh = 2, 16, 2048, 64
KV = 4096
HID = H * Dh
NBLK = S // 128          # 16 query blocks per batch
POS = 4                  # block positions per core
CORES = 8
SCALE = 1.0 / np.sqrt(Dh).astype(np.float32)
GROUP = 2                # score chunks per exp call (PSUM banks)

_prog_cache = {}


def _limits(m):
    # L(r) for all rows of a batch
    r = np.arange(S)
    return np.minimum(KV, np.maximum(r, m - 1) + (KV - S) + 1)


def _build_schedule(m):
    L = _limits(m)
    Lmin = np.array([L[128 * k] for k in range(NBLK)])
    Lmax = np.array([L[128 * k + 127] for k in range(NBLK)])
    full_until = Lmin // 128 - 1          # chunks 0..full_until fully allowed
    active_until = (Lmax + 127) // 128 - 1
    blk = lambda j, p: 4 * p + j          # core j, position p -> block
    union_until = [max(active_until[blk(j, p)] for j in range(4)) for p in range(POS)]
    n_chunks = max(union_until) + 1

    def p0(c):
        for p in range(POS):
            if union_until[p] >= c:
                return p
        return POS

    qoff = [128 * p0(c) for c in range(n_chunks)]
    regions = []   # (c, p) needing a data mask
    for c in range(n_chunks):
        for p in range(p0(c), POS):
            if any(c > full_until[blk(j, p)] for j in range(4)):
                regions.append((c, p))
    masks = np.empty((4, max(1, len(regions)), 128, 128), np.float32)
    masks[:] = 1.0
    for mi, (c, p) in enumerate(regions):
        for j in range(4):
            rows = 128 * blk(j, p) + np.arange(128)
            cols = 128 * c + np.arange(128)
            # score tiles are transposed: [t_local (partition), r_local (free)]
            masks[j, mi] = (cols[:, None] < L[rows][None, :]).astype(np.float32)
    groups = [list(range(g0, min(g0 + GROUP, n_chunks)))
              for g0 in range(0, n_chunks, GROUP)]
    return dict(n_chunks=n_chunks, qoff=qoff, regions=regions, masks=masks,
                groups=groups, blocks=[[blk(j, p) for p in range(POS)] for j in range(4)])


def _build_program(sched, n_masks):
    import concourse.bacc as bacc
    import concourse.mybir as mybir
    import concourse.tile as tile

    f32 = mybir.dt.float32
    bfl = mybir.dt.float16
    Exp = mybir.ActivationFunctionType.Exp
    n_chunks = sched["n_chunks"]
    qoff = sched["qoff"]
    groups = sched["groups"]
    reg_idx = {cp: i for i, cp in enumerate(sched["regions"])}

    nc = bacc.Bacc("TRN2", target_bir_lowering=False, debug=False)
    # head pairs (2p, 2p+1) stacked on the partition axis for QK row-packing
    kt_d = nc.dram_tensor("kt", [H // 2, 128, KV], bfl, kind="ExternalInput").ap()
    v_d = nc.dram_tensor("v", [H, 128, KV // 128, Dh + 1], bfl, kind="ExternalInput").ap()
    qt_d = nc.dram_tensor("qt", [H // 2, 128, 512], bfl, kind="ExternalInput").ap()
    wt_d = nc.dram_tensor("wt", [HID, HID], bfl, kind="ExternalInput").ap()
    bb_d = nc.dram_tensor("bb", [128, HID], f32, kind="ExternalInput").ap()
    mk_d = nc.dram_tensor("masks", [n_masks, 128, 128], bfl, kind="ExternalInput").ap()
    out_d = nc.dram_tensor("out", [512, HID], f32, kind="ExternalOutput").ap()

    with tile.TileContext(nc) as tc:
        with (
            tc.tile_pool(name="consts", bufs=1) as consts,
            tc.tile_pool(name="attn", bufs=1) as attn_pool,
        ):
            bb_sb = consts.tile([128, HID], f32)
            nc.sync.dma_start(out=bb_sb, in_=bb_d)
            mk_sb = consts.tile([128, n_masks, 128], bfl)
            nc.sync.dma_start(out=mk_sb, in_=mk_d.rearrange("n p t -> p n t"))
            # out accumulator, bias-initialized; heads add their projection here
            ob_acc = attn_pool.tile([128, 4, HID], f32)
            warm_d = nc.dram_tensor("warm", [1, 1], f32, kind="ExternalOutput").ap()

            with (
                tc.tile_pool(name="kv", bufs=2) as kv_pool,
                tc.tile_pool(name="pt", bufs=3) as pt_pool,
                tc.tile_pool(name="wt", bufs=3) as wt_pool,
                tc.tile_pool(name="small", bufs=2) as small,
                tc.tile_pool(name="at", bufs=2) as at_pool,
                tc.tile_pool(name="ps_s", bufs=2, space="PSUM") as ps_s,
                tc.tile_pool(name="ps_av", bufs=2, space="PSUM") as ps_av,
                tc.tile_pool(name="ps_o", bufs=2, space="PSUM") as ps_o,
            ):
                # ACT table preload + PE warmup chain (runs while DMAs load)
                wtile = small.tile([1, 8], f32, tag="wtile")
                nc.vector.memset(wtile, 0.0)
                nc.scalar.activation(wtile, wtile, Exp, scale=1.0)
                po_w = ps_o.tile([128, 512], f32, tag="po")
                for i in range(48):
                    nc.tensor.matmul(po_w[:, 0:128], mk_sb[:, 0, :], mk_sb[:, 0, :],
                                     start=(i == 0), stop=(i == 47))
                nc.sync.dma_start(out=warm_d, in_=wtile[0:1, 0:1])
                for qs in range(4):
                    nc.vector.tensor_copy(ob_acc[:, qs, :], bb_sb)

                for h in range(H):
                    pr, hi = h // 2, h % 2
                    kt_h = kv_pool.tile([Dh, KV], bfl, tag="kt")
                    nc.sync.dma_start(out=kt_h, in_=kt_d[pr, 64 * hi:64 * (hi + 1), :])
                    qt_h = kv_pool.tile([Dh, 512], bfl, tag="qt")
                    nc.sync.dma_start(out=qt_h, in_=qt_d[pr, 64 * hi:64 * (hi + 1), :])
                    v_h = kv_pool.tile([128, KV // 128, Dh + 1], bfl, tag="v")
                    nc.sync.dma_start(out=v_h, in_=v_d[h])
                    wt_h = wt_pool.tile([Dh, HID], bfl, tag="wt")
                    nc.sync.dma_start(out=wt_h, in_=wt_d[Dh * h:Dh * (h + 1), :])

                    av = ps_av.tile([Dh + 1, 512], f32, tag="av")
                    for cs in groups:
                        sg = ps_s.tile([128, GROUP, 512], f32, tag="sg")
                        for ci, c in enumerate(cs):
                            nc.tensor.matmul(
                                sg[:, ci, qoff[c]:512],
                                kt_h[:, 128 * c:128 * (c + 1)],
                                qt_h[:, qoff[c]:512],
                                start=True, stop=True,
                            )
                        ptg = pt_pool.tile([128, GROUP, 512], bfl, tag="ptg")
                        ci0 = 0
                        while ci0 < len(cs):
                            ci1 = ci0
                            while ci1 < len(cs) and qoff[cs[ci1]] == qoff[cs[ci0]]:
                                ci1 += 1
                            qg = qoff[cs[ci0]]
                            nc.scalar.activation(
                                ptg[:, ci0:ci1, qg:512], sg[:, ci0:ci1, qg:512],
                                Exp, scale=float(SCALE),
                            )
                            ci0 = ci1
                        for ci, c in enumerate(cs):
                            for p in range(POS):
                                mi = reg_idx.get((c, p))
                                if mi is not None:
                                    nc.vector.tensor_mul(
                                        ptg[:, ci, 128 * p:128 * (p + 1)],
                                        ptg[:, ci, 128 * p:128 * (p + 1)],
                                        mk_sb[:, mi, :],
                                    )
                            nc.tensor.matmul(
                                av[:, qoff[c]:512],
                                v_h[:, c, :],
                                ptg[:, ci, qoff[c]:512],
                                start=(c == 0), stop=(c == n_chunks - 1),
                            )
                    den = small.tile([1, 512], f32, tag="den")
                    nc.vector.tensor_copy(den, av[Dh:Dh + 1, :])
                    recip = small.tile([1, 512], f32, tag="recip")
                    nc.vector.reciprocal_approx_fast(recip, den)
                    mult = small.tile([Dh, 512], f32, tag="mult")
                    nc.gpsimd.partition_broadcast(mult, recip)
                    attnT = at_pool.tile([Dh, 512], bfl, tag="attnT")
                    nc.vector.tensor_mul(attnT, av[0:Dh, :], mult)
                    # projection contribution of this head, accumulated on DVE
                    for qs in range(4):
                        for jh in range(2):
                            po = ps_o.tile([128, 512], f32, tag="po")
                            nc.tensor.matmul(
                                po, attnT[:, 128 * qs:128 * (qs + 1)],
                                wt_h[:, 512 * jh:512 * (jh + 1)],
                                start=True, stop=True,
                            )
                            nc.vector.tensor_add(
                                ob_acc[:, qs, 512 * jh:512 * (jh + 1)],
                                ob_acc[:, qs, 512 * jh:512 * (jh + 1)], po)
                for qs in range(4):
                    nc.sync.dma_start(out=out_d[128 * qs:128 * (qs + 1), :],
                                      in_=ob_acc[:, qs, :])

    nc.compile()
    return nc


def _prepare(q, k, v, m, W, b):
    sched = _build_schedule(m)
    kt = np.ascontiguousarray(k.transpose(0, 1, 3, 2).astype(MM_NP))   # [B,H,64,KV]
    kt = kt.reshape(B, H // 2, 128, KV)
    vr = v.reshape(B, H, KV // 128, 128, Dh).transpose(0, 1, 3, 2, 4)
    v_aug = np.empty((B, H, 128, KV // 128, Dh + 1), MM_NP)
    v_aug[..., :Dh] = vr.astype(MM_NP)
    v_aug[..., Dh] = 1.0
    wt = np.ascontiguousarray(W.T.astype(MM_NP))
    bb = np.ascontiguousarray(np.broadcast_to(b, (128, HID)).astype(np.float32))
    in_maps = []
    for core in range(CORES):
        bidx, j = core // 4, core % 4
        rows = np.concatenate([128 * kblk + np.arange(128) for kblk in sched["blocks"][j]])
        qt = np.ascontiguousarray(
            q[bidx][:, rows, :].transpose(0, 2, 1).astype(MM_NP)).reshape(H // 2, 128, 512)
        in_maps.append({
            "kt": kt[bidx], "v": v_aug[bidx], "qt": qt, "wt": wt, "bb": bb,
            "masks": np.ascontiguousarray(sched["masks"][j].astype(MM_NP)),
        })
    return sched, in_maps


def kernel(q, k, v, memory_length, W, b, _trace=False, _trace_cores=None):
    from concourse.bass_utils import run_bass_kernel_spmd

    q = np.asarray(q, np.float32)
    k = np.asarray(k, np.float32)
    v = np.asarray(v, np.float32)
    W = np.asarray(W, np.float32)
    b = np.asarray(b, np.float32)
    m = int(memory_length)

    sched, in_maps = _prepare(q, k, v, m, W, b)
    key = (m, sched["masks"].shape[1])
    if key not in _prog_cache:
        _prog_cache[key] = _build_program(sched, sched["masks"].shape[1])
    nc = _prog_cache[key]

    res = run_bass_kernel_spmd(
        nc, in_maps, list(range(CORES)),
        trace=_trace, trace_cores=_trace_cores,
    )
    out = np.empty((B, S, HID), np.float32)
    for core in range(CORES):
        bidx, j = core // 4, core % 4
        oc = res.results[core]["out"]
        for p, kblk in enumerate(sched["blocks"][j]):
            out[bidx, 128 * kblk:128 * (kblk + 1)] = oc[128 * p:128 * (p + 1)]

    L = _limits(m)
    mask2d = (np.arange(KV)[None, :] >= L[:, None]).astype(np.float32)
    mask_full = np.broadcast_to(mask2d, (B, H, S, KV))
    if _trace:
        return (out, mask_full), res
    return out, mask_full
